# revision 2
# baseline (speedup 1.0000x reference)
"""Trainium2 Bass kernel for nn_EntityEmbedding (embedding lookup + mean pool).

reference:  emb = weights[x]            # [B, I, H] gather from [V, H] table
            out = emb.mean(axis=1)      # [B, H]

Shapes (hardcoded): x [16384, 50] int indices into V=1,000,000 rows; weights
[1000000, 64] f32; out [16384, 64] f32.

Strategy: replicate the 256 MB table in every core's HBM; data-parallel
shard the batch 8 ways (2048 rows/core). Per core:
  - gpsimd loads all indices into SBUF as [p][t][i] (batch row = t*128+p),
  - gpsimd issues 800 indirect DMAs (one per batch-tile x slot) gathering
    128 table rows each (HW consumes exactly one offset per partition per
    indirect DMA; multi-offset APs gather consecutive rows, and reshaped
    dst APs misread or crash - verified on HW), ring of 4 tile buffers,
  - DVE reduces each gathered [128, 50*64] tile over the 50 slots and
    scales by 1/50,
  - sync engine (HWDGE) stores each reduced tile as soon as DVE finishes.

The gpsimd stream is the bottleneck: measured ~1.04us marginal per
indirect DMA (SWDGE descriptor generation, serialized on the Q7 cluster,
queue pair hardwired to 0 in ucode; extended-inst bulk dma_gather is
excluded from this bedrock image) -> ~830us/core floor for 800 gathers.
Transfers (26 MB/core of random 256B reads) hide entirely under it.

Single-wait discipline: every instruction carries at most one semaphore
wait (the walrus codegen path rejects more).
"""
import numpy as np

import concourse.bass as bass
import concourse.bacc as bacc
import concourse.mybir as mybir
from concourse.bass_utils import run_bass_kernel_spmd

P = 128
B_FULL = 16384
I = 50
H = 64
V = 1000000
N_CORES = 8
B_CORE = B_FULL // N_CORES
NT = B_CORE // P           # 16 batch tiles per core
G_BUFS = 4                 # gathered-tile ring


def _build_nc():
    nc = bacc.Bacc(None)
    x = nc.declare_dram_parameter("x", [B_CORE, I], mybir.dt.int32, isOutput=False)
    w = nc.declare_dram_parameter("weights", [V, H], mybir.dt.float32, isOutput=False)
    out = nc.declare_dram_parameter("out", [B_CORE, H], mybir.dt.float32, isOutput=True)

    TILE_F = I * H  # 3200 f32 per partition per gathered tile
    with (
        nc.sbuf_tensor([P, NT * I], mybir.dt.int32) as idx_sb,
        nc.sbuf_tensor([P, G_BUFS * TILE_F], mybir.dt.float32) as g_sb,
        nc.sbuf_tensor([P, H], mybir.dt.float32) as s_sb,
        nc.sbuf_tensor([P, NT * H], mybir.dt.float32) as obig,
        nc.semaphore("dma") as dma,
        nc.semaphore("dvs") as dvs,
        nc.semaphore("st") as st,
        nc.Block() as block,
    ):
        @block.gpsimd
        def _(gpsimd):
            gpsimd.dma_start(
                idx_sb[:].rearrange("p (t i) -> p t i", t=NT),
                x[:].rearrange("(t p) i -> p t i", p=P),
            ).then_inc(dma, 16)
            gpsimd.wait_ge(dma, 16)
            for t in range(NT):
                if t >= G_BUFS:
                    # ring slot free once DVE finished tile t - G_BUFS
                    gpsimd.wait_ge(dvs, t - G_BUFS + 1)
                base = (t % G_BUFS) * TILE_F
                for j in range(I):
                    gpsimd.indirect_dma_start(
                        out=g_sb[:, base + j * H: base + (j + 1) * H],
                        out_offset=None,
                        in_=w[:],
                        in_offset=bass.IndirectOffsetOnAxis(
                            ap=idx_sb[:, t * I + j: t * I + j + 1], axis=0
                        ),
                    ).then_inc(dma, 16)
            gpsimd.wait_ge(dvs, NT)

        @block.sync
        def _(sync):
            # store each reduced tile as soon as DVE finishes it
            # (batch row = t*128 + p, so tile t's rows are contiguous)
            for t in range(NT):
                sync.wait_ge(dvs, t + 1)
                sync.dma_start(
                    out[t * P:(t + 1) * P, :],
                    obig[:, t * H: (t + 1) * H],
                ).then_inc(st, 16)
            sync.wait_ge(st, 16 * NT)

        @block.vector
        def _(vector):
            for t in range(NT):
                # all 50 gathers of tile t complete
                vector.wait_ge(dma, 16 * (1 + I * (t + 1)))
                base = (t % G_BUFS) * TILE_F
                g3 = g_sb[:, base: base + TILE_F].rearrange(
                    "p (i h) -> p h i", i=I
                )
                nc.vector.tensor_reduce(
                    s_sb[:], g3, axis=mybir.AxisListType.X, op=mybir.AluOpType.add
                )
                nc.vector.tensor_scalar_mul(
                    obig[:, t * H: (t + 1) * H], s_sb[:], 1.0 / I
                ).then_inc(dvs, 1)

    nc.compile()
    return nc


def _run(x, weights, trace=False):
    x = np.ascontiguousarray(np.asarray(x), dtype=np.int32)
    weights = np.ascontiguousarray(np.asarray(weights), dtype=np.float32)
    assert x.shape == (B_FULL, I) and weights.shape == (V, H)

    nc = _build_nc()
    in_maps = [
        {"x": x[c * B_CORE:(c + 1) * B_CORE], "weights": weights}
        for c in range(N_CORES)
    ]
    res = run_bass_kernel_spmd(nc, in_maps, list(range(N_CORES)), trace=trace)
    out = np.concatenate([res.results[c]["out"] for c in range(N_CORES)], axis=0)
    return out, res


def kernel(x, weights):
    out, _ = _run(x, weights, trace=False)
    return out
